# revision 54
# baseline (speedup 1.0000x reference)
"""AnomalyScorer Trainium2 kernel (8 NeuronCores, SPMD edge-parallel), v7.

Strategy (fp8 tables + cached-norms identity + paired u-gathers +
feature-split products across DVE/GpSimd):
  - Host folds per-feature scales a/b into fp8(e4m3) tables (ha = h*a,
    hb = h*b) and precomputes per-node squared norms p = ||ha||^2,
    q = ||hb||^2 of the quantized rows, using the cached-norms identity
        ||ha_u + hb_v||^2 = p[u] + q[v] + 2*dot(ha_u, hb_v)
    so the device work per edge is one 256-feature dot product. The
    identity is exact for the quantized tables; p[u]+q[v]-mu is
    precomputed per edge (host O(E) scalar work) and streamed in.
  - fp8 rows are 256B/edge/side; gathered as f32 words (the DMA cost
    model charges per element) the whole gather stream is ~31.6us/core,
    half of what bf16 rows would cost.
  - Edges are globally sorted by u and RANGE-sharded across the 8 cores
    (37500 each, padded to 37888), so each core's unique u count is
    ~12.5K and consecutive compacted u-ids differ by 0 or 1: a PAIR of
    edges is served by one 512B descriptor from a host-built pair table
    (row 2k = [u_k|u_k], 2k+1 = [u_k|u_k+1]; index 2*id+delta), halving
    u-side SWDGE descriptor-generation on GpSimd. The v index stream is
    permuted so pair (2j,2j+1) lands on partition j%128, columns
    2(j//128)+{0,1}, making the flat tiles layout-identical on both
    sides (the compute never sees the pairing).
  - Gathers use prepare_only+trigger_dma with a 3-deep window pipeline
    and an enlarged SWDGE descriptor ring; each window's gathers are
    emitted BEFORE the previous window's compute so transfers start as
    early as possible, and the window sizes ramp up/down (ladder) to
    shorten pipeline fill and drain. RAW syncs from the DMA to the
    product engines are wired manually via per-window semaphores (the
    tile framework does not connect prep-only gathers to their
    consumers).
  - Per 16-column piece, the elementwise product (fp8 in, bf16 out) is
    FEATURE-SPLIT: DVE computes features [0,160) while GpSimd computes
    [160,256) of the same piece concurrently - both engines stay busy
    on every piece with no assignment imbalance. The TensorEngine then
    pre-sums each column into 8 partial sums in PSUM via accumulated
    identity-matmuls, and the final 8-wide reduce runs on DVE (batched
    tensor_reduce) or ACT (per-column activation accum, ~35% of pieces)
    into a persistent [128, 296] f32 norm tile.
  - Epilogue (once): DVE scalar_tensor_tensor forms 2*dot + (p+q-mu),
    ACT applies sigmoid, DVE multiplies by the edge weights, one DMA out.
  - A bf16-window variant (dtype 16 in ANOM_WINDOWS) is kept for
    experimentation but unused by default.
"""

import os

import numpy as np

N_CORES = 8
N_NODES = 100000
D = 256
E_TOTAL = 300000
EPC = E_TOTAL // N_CORES          # 37500 edges per core
EPAD = 37888                      # padded edges per core (296*128, mult of 256)
T = EPAD // 128                   # 296 columns of 128 edges
NU_PAD = 32768                    # table rows (int16 id space)
# "edges:dtype" per window; dtype 8 = fp8 rows, 16 = bf16 rows
WSPEC = os.environ.get(
    "ANOM_WINDOWS",
    "2048:8,4096:8,6144:8,6144:8,6144:8,6144:8,4608:8,2560:8",
)
WINDOWS = [(int(a), int(b)) for a, b in (w.split(":") for w in WSPEC.split(","))]
assert sum(w for w, _ in WINDOWS) == EPAD
assert all(w % 256 == 0 for w, _ in WINDOWS)
KU8 = max((w for w, d in WINDOWS if d == 8), default=256) // 256   # pair cols
KV8 = max((w for w, d in WINDOWS if d == 8), default=128) // 128
KU16 = max((w for w, d in WINDOWS if d == 16), default=256) // 256
KV16 = max((w for w, d in WINDOWS if d == 16), default=128) // 128
BETA = 1.0
MU = 0.5
PIECE = int(os.environ.get("ANOM_PIECE", "16"))  # columns per piece
S = int(os.environ.get("ANOM_S", "8"))   # PE pre-sum width per column
# feature split for fp8 products: DVE computes features [0, FDVE),
# GpSimd computes [FDVE, 256) of the SAME piece concurrently
FDVE = int(os.environ.get("ANOM_FDVE", "160"))
# late windows: transfers are done, GpSimd has slack -> give it more features
FDVE_LATE = int(os.environ.get("ANOM_FDVE_LATE", "160"))
LATE_WINS = int(os.environ.get("ANOM_LATE_WINS", "2"))
ACT_FRAC = float(os.environ.get("ANOM_ACT_FRAC", "0.35"))
DRAIN_DVE = int(os.environ.get("ANOM_DRAIN_DVE", "2"))

_cache = {}


def _build_graph():
    import concourse.bacc as bacc
    import concourse.tile as tile
    from concourse import mybir
    from concourse.masks import make_identity

    f32 = mybir.dt.float32
    i16 = mybir.dt.int16
    fp8 = mybir.dt.float8e4
    bf16 = mybir.dt.bfloat16

    nc = bacc.Bacc(dynamic_dma_scratch_size=int(os.environ.get("ANOM_SCRATCH", "49152")))
    tab_u8 = nc.declare_dram_parameter("tab_u8", [NU_PAD, D // 2], f32, isOutput=False)
    tab_v8 = nc.declare_dram_parameter("tab_v8", [NU_PAD, D // 4], f32, isOutput=False)
    tab_u16 = nc.declare_dram_parameter("tab_u16", [NU_PAD, D], f32, isOutput=False)
    tab_v16 = nc.declare_dram_parameter("tab_v16", [NU_PAD, D // 2], f32, isOutput=False)
    iu = nc.declare_dram_parameter("iu", [128, EPAD // 32], i16, isOutput=False)
    iv = nc.declare_dram_parameter("iv", [128, EPAD // 16], i16, isOutput=False)
    pqm = nc.declare_dram_parameter("pqm", [128, T], f32, isOutput=False)
    ws = nc.declare_dram_parameter("ws", [128, T], f32, isOutput=False)
    out = nc.declare_dram_parameter("out", [128, T], f32, isOutput=True)

    AHEAD = int(os.environ.get("ANOM_AHEAD", "1"))
    # idx cols needed for the windows whose preps may run before the bulk
    # idx load completes (covers prefetch depth + pipeline skew)
    W0WINS = int(os.environ.get("ANOM_W0WINS", "2"))
    W0E = sum(w for w, _ in WINDOWS[: max(1, AHEAD, W0WINS)])
    W0CU = W0E // 32
    W0CV = W0E // 16

    with tile.TileContext(nc) as tc:
        with (
            tc.tile_pool(name="io", bufs=1) as io,
            tc.tile_pool(name="wp", bufs=int(os.environ.get("ANOM_BUFS", "3"))) as wp,
            tc.tile_pool(name="pp", bufs=int(os.environ.get("ANOM_PBUFS", "4"))) as pp,
            tc.tile_pool(name="ps", bufs=int(os.environ.get("ANOM_PSBUFS", "8")), space="PSUM") as psp,
        ):
            iu_t = io.tile([128, EPAD // 32], i16)
            iv_t = io.tile([128, EPAD // 16], i16)
            nc.sync.dma_start(out=iu_t[:, :W0CU], in_=iu[:, :W0CU])
            nc.sync.dma_start(out=iv_t[:, :W0CV], in_=iv[:, :W0CV])

            ident = io.tile([128, 128], bf16)
            make_identity(nc, ident[:])
            pqm_t = io.tile([128, T], f32)
            ws_t = io.tile([128, T], f32)
            norm_t = io.tile([128, T], f32)
            sq = io.tile([128, S], f32)

            n_pieces = sum(
                (wn // 128 + PIECE - 1) // PIECE for wn, _ in WINDOWS
            )

            woff = []
            acc = 0
            for wn, _ in WINDOWS:
                woff.append(acc)
                acc += wn

            gathered = {}

            def emit_gathers(wi):
                """Prep+trigger both gathers of window wi; returns tiles+sems."""
                wn, wdt = WINDOWS[wi]
                w0 = woff[wi]
                kk = wn // 128
                if wdt == 8:
                    tu = wp.tile([128, KU8, D // 2], f32, tag="tu8")
                    tv = wp.tile([128, KV8, D // 4], f32, tag="tv8")
                    t_u, t_v = tab_u8, tab_v8
                    eu, ev = D // 2, D // 4
                    cdt = fp8
                else:
                    tu = wp.tile([128, KU16, D], f32, tag="tu16")
                    tv = wp.tile([128, KV16, D // 2], f32, tag="tv16")
                    t_u, t_v = tab_u16, tab_v16
                    eu, ev = D, D // 2
                    cdt = bf16
                # prepare_only + trigger: desc-gen holds Pool only for the
                # SWDGE time; the transfer itself runs on the DMA engines
                sem_u = nc.alloc_semaphore(f"g_u{wi}")
                sem_v = nc.alloc_semaphore(f"g_v{wi}")
                nc.gpsimd.dma_gather(
                    tu[:, : wn // 256, :], t_u[:],
                    iu_t[:, w0 // 32 : (w0 + wn) // 32],
                    wn // 2, wn // 2, eu, single_packet=False,
                    prepare_only=True, sem=sem_u,
                )
                nc.gpsimd.trigger_dma(count=None)
                nc.gpsimd.dma_gather(
                    tv[:, :kk, :], t_v[:],
                    iv_t[:, w0 // 16 : (w0 + wn) // 16],
                    wn, wn, ev, single_packet=False,
                    prepare_only=True, sem=sem_v,
                )
                nc.gpsimd.trigger_dma(count=None)
                gathered[wi] = (tu, tv, sem_u, sem_v, cdt)

            piece_i = 0
            act_acc = float(os.environ.get("ANOM_ACT_SEED", "0.0"))
            emit_gathers(0)
            for wi in range(1, min(AHEAD, len(WINDOWS))):
                emit_gathers(wi)
            for wi, (wn, wdt) in enumerate(WINDOWS):
                w0 = woff[wi]
                kk = wn // 128
                if wi == 0:
                    # deferred bulk loads: slot in behind the first gathers
                    nc.sync.dma_start(out=iu_t[:, W0CU:], in_=iu[:, W0CU:])
                    nc.sync.dma_start(out=iv_t[:, W0CV:], in_=iv[:, W0CV:])
                    nc.sync.dma_start(out=pqm_t[:], in_=pqm[:])
                    nc.sync.dma_start(out=ws_t[:], in_=ws[:])
                # queue a later window's gathers before this window's
                # compute so its transfers start as early as possible
                if wi + AHEAD < len(WINDOWS):
                    emit_gathers(wi + AHEAD)
                tu, tv, sem_u, sem_v, cdt = gathered.pop(wi)
                # the tile framework does not wire RAW syncs from prep-only
                # gathers to their consumers; wait on the DMA sems manually
                # on each product engine before this window's first product
                nc.vector.wait_ge(sem_u, 16)
                nc.vector.wait_ge(sem_v, 16)
                nc.gpsimd.wait_ge(sem_u, 16)
                nc.gpsimd.wait_ge(sem_v, 16)
                # both flat views are [128, kk, 256] in the window dtype
                tub = tu[:].bitcast(cdt).rearrange("p a (c e) -> p (a c) e", c=2)
                tvb = tv[:].bitcast(cdt)
                base = w0 // 128
                for s0 in range(0, kk, PIECE):
                    s1 = min(s0 + PIECE, kk)
                    npc = s1 - s0
                    piece_i += 1
                    prod = pp.tile([128, PIECE, D], bf16, tag="prod")
                    fd = FDVE_LATE if wi >= len(WINDOWS) - LATE_WINS else FDVE
                    if wdt == 8:
                        nc.vector.tensor_tensor(
                            out=prod[:, :npc, :fd], in0=tub[:, s0:s1, :fd],
                            in1=tvb[:, s0:s1, :fd], op=mybir.AluOpType.mult,
                        )
                        nc.gpsimd.tensor_tensor(
                            out=prod[:, :npc, fd:], in0=tub[:, s0:s1, fd:],
                            in1=tvb[:, s0:s1, fd:], op=mybir.AluOpType.mult,
                        )
                    else:
                        nc.vector.tensor_tensor(
                            out=prod[:, :npc, :], in0=tub[:, s0:s1, :],
                            in1=tvb[:, s0:s1, :], op=mybir.AluOpType.mult,
                        )
                    qs = psp.tile([128, PIECE, S], f32, tag="qs")
                    nq = D // S
                    for q in range(nq):
                        nc.tensor.matmul(
                            out=qs[:, :npc, :], lhsT=ident[:],
                            rhs=prod[:, :npc, q * S : q * S + S],
                            start=(q == 0), stop=(q == nq - 1),
                        )
                    cols = slice(base + s0, base + s1)
                    act_acc += ACT_FRAC
                    on_act = act_acc >= 1.0 and piece_i <= n_pieces - DRAIN_DVE
                    if act_acc >= 1.0:
                        act_acc -= 1.0
                    if on_act:
                        for j in range(npc):
                            nc.scalar.activation(
                                out=sq[:], in_=qs[:, j, :],
                                func=mybir.ActivationFunctionType.Copy,
                                accum_out=norm_t[:, base + s0 + j : base + s0 + j + 1],
                            )
                    else:
                        nc.vector.tensor_reduce(
                            out=norm_t[:, cols], in_=qs[:, :npc, :],
                            axis=mybir.AxisListType.X, op=mybir.AluOpType.add,
                        )
                w0 += wn
            assert w0 == EPAD

            # split epilogue: the first chunk of norm columns is complete
            # long before the last window, so its sigmoid/weight/store runs
            # mid-stream and only the tail chunk sits in the drain
            z = io.tile([128, T], f32)
            sig = io.tile([128, T], f32)
            out_t = io.tile([128, T], f32)
            esplit = [int(x) for x in os.environ.get("ANOM_ESPLIT", "296").split(",")]
            e0 = 0
            for e1 in esplit:
                nc.vector.scalar_tensor_tensor(
                    out=z[:, e0:e1], in0=norm_t[:, e0:e1], scalar=2.0,
                    in1=pqm_t[:, e0:e1],
                    op0=mybir.AluOpType.mult, op1=mybir.AluOpType.add,
                )
                nc.scalar.activation(
                    out=sig[:, e0:e1], in_=z[:, e0:e1],
                    func=mybir.ActivationFunctionType.Sigmoid, scale=BETA,
                )
                nc.vector.tensor_tensor(
                    out=out_t[:, e0:e1], in0=sig[:, e0:e1],
                    in1=ws_t[:, e0:e1], op=mybir.AluOpType.mult,
                )
                nc.sync.dma_start(out=out[:, e0:e1], in_=out_t[:, e0:e1])
                e0 = e1

    nc.finalize()
    return nc


def _wrap_idx(idx16):
    """int16 [n] -> [128, n//16]; element j at [j%16, j//16], tiled x8."""
    n = idx16.shape[0]
    w = idx16.reshape(n // 16, 16).T
    return np.ascontiguousarray(np.tile(w, (8, 1)))


def _slot_maps():
    e = np.arange(EPAD)
    j, s = e // 2, e % 2
    return j % 128, 2 * (j // 128) + s


def _edge_dtype_mask():
    """Per edge-slot: True if its window uses fp8."""
    m = np.zeros(EPAD, bool)
    w0 = 0
    for wn, wdt in WINDOWS:
        if wdt == 8:
            m[w0 : w0 + wn] = True
        w0 += wn
    return m


def _pair_table(rows, dtype, nu):
    """Pair table: row 2k = [row_k|row_k], 2k+1 = [row_k|row_k+1]."""
    tab = np.zeros((NU_PAD, 2 * D), dtype=dtype)
    tab[0 : 2 * nu : 2, :D] = rows
    tab[0 : 2 * nu : 2, D:] = rows
    tab[1 : 2 * nu : 2, :D] = rows
    tab[1 : 2 * nu - 1 : 2, D:] = rows[1:]
    return tab


def _prepare_inputs(h, us, vs, ws, a, b):
    import ml_dtypes

    e4 = ml_dtypes.float8_e4m3fn
    bf = ml_dtypes.bfloat16
    h = np.asarray(h, dtype=np.float32)
    a = np.asarray(a, dtype=np.float32)
    b = np.asarray(b, dtype=np.float32)
    us = np.asarray(us).astype(np.int64, copy=False)
    vs = np.asarray(vs).astype(np.int64, copy=False)
    w = np.asarray(ws, dtype=np.float32)

    ha8 = (h * a[None, :]).astype(e4)
    hb8 = (h * b[None, :]).astype(e4)
    ha16 = (h * a[None, :]).astype(bf)
    hb16 = (h * b[None, :]).astype(bf)

    def norms(x):
        xf = x.astype(np.float32)
        return np.einsum("nd,nd->n", xf, xf, dtype=np.float64).astype(np.float32)

    p8, q8, p16, q16 = norms(ha8), norms(hb8), norms(ha16), norms(hb16)

    order = np.argsort(us, kind="stable")
    pp_, tt_ = _slot_maps()
    is8 = _edge_dtype_mask()

    in_maps = []
    for c in range(N_CORES):
        idx = order[c * EPC : (c + 1) * EPC]
        u = np.concatenate([us[idx], np.zeros(EPAD - EPC, np.int64)])
        v = np.concatenate([vs[idx], np.zeros(EPAD - EPC, np.int64)])
        wc = np.concatenate([w[idx], np.zeros(EPAD - EPC, np.float32)])

        uu, iu_ids = np.unique(u, return_inverse=True)
        vv, iv_ids = np.unique(v, return_inverse=True)
        assert EPC % 2 == 0
        nu, nv = len(uu), len(vv)
        if 2 * nu > NU_PAD or nv > NU_PAD:
            raise RuntimeError(f"core {c}: table overflow nu={nu} nv={nv}")

        tab_u8 = _pair_table(ha8[uu], e4, nu)
        tab_u16 = _pair_table(ha16[uu], bf, nu)
        tab_v8 = np.zeros((NU_PAD, D), dtype=e4)
        tab_v8[:nv] = hb8[vv]
        tab_v16 = np.zeros((NU_PAD, D), dtype=bf)
        tab_v16[:nv] = hb16[vv]

        ids0, ids1 = iu_ids[0::2], iu_ids[1::2]
        delta = ids1 - ids0
        if not np.all((delta == 0) | (delta == 1)):
            raise RuntimeError(f"core {c}: non-adjacent pair deltas")
        pair_idx = (2 * ids0 + delta).astype(np.int16)

        pos = np.empty(EPAD, np.int64)
        pos[:] = 0
        e = np.arange(EPAD)
        j, s = e // 2, e % 2
        pos_e = (j // 128) * 256 + s * 128 + (j % 128)
        iv_stream = np.empty(EPAD, np.int16)
        iv_stream[pos_e] = iv_ids.astype(np.int16)

        pqm = np.where(is8, p8[u] + q8[v], p16[u] + q16[v]) - MU
        pqm[EPC:] = 0.0

        def to_slots(arr):
            m = np.zeros((128, T), np.float32)
            m[pp_, tt_] = arr
            return m

        in_maps.append(
            {
                "tab_u8": tab_u8.view(np.float32),
                "tab_v8": tab_v8.view(np.float32),
                "tab_u16": tab_u16.view(np.float32),
                "tab_v16": tab_v16.view(np.float32),
                "iu": _wrap_idx(pair_idx),
                "iv": _wrap_idx(iv_stream),
                "pqm": to_slots(pqm),
                "ws": to_slots(wc),
            }
        )
    return in_maps, order


def kernel(h, us, vs, ws, a, b):
    from concourse.bass_utils import run_bass_kernel_spmd

    if "nc" not in _cache:
        _cache["nc"] = _build_graph()
    nc = _cache["nc"]

    in_maps, order = _prepare_inputs(h, us, vs, ws, a, b)
    res = run_bass_kernel_spmd(nc, in_maps, core_ids=list(range(N_CORES)))
    _cache["last_results"] = res

    pp_, tt_ = _slot_maps()
    full = np.empty(E_TOTAL, np.float32)
    for c in range(N_CORES):
        o = res.results[c]["out"]                 # [128, T]
        full[order[c * EPC : (c + 1) * EPC]] = o[pp_[:EPC], tt_[:EPC]]
    return full
